# revision 1
# baseline (speedup 1.0000x reference)
"""Trainium2 Bass kernel: GPT-2-style causal multi-head attention.

Problem: B=4, S=2048, D=1024, H=16 heads (head_dim 64), fp32.
  q/k/v = x @ W{q,k,v} + b{q,k,v}; causal softmax attention; out = attn_out @ Wo + bo.

Sharding (8 cores): tensor-parallel over heads - each core owns 2 heads
(128 feature dims). Wq/Wk/Wv column-sliced, Wo row-sliced per core. Each core
computes a partial o_proj output (transposed, [D, B*S]); the host sums the 8
partials, transposes, and adds bo.

Layout strategy on-chip: everything is kept transposed ([feature, seq]) so that
all matmul contractions have their contraction dim on SBUF partitions:
  x^T (via PE transpose) -> q^T/k^T/v^T = W^T x^T -> S^T = K^T^T... scores
  computed as S^T[k, q] tiles -> exp on ACT -> P^T -> out^T = V^T-ext @ P^T
  (with an appended ones column producing the softmax denominators) ->
  normalize -> o_proj out^T = Wo^T attnout^T.
"""

import sys
import os

sys.path.insert(0, "/opt/trn_rl_repo")

import numpy as np

import concourse.bass as bass
import concourse.bacc as bacc
import concourse.tile as tile
import concourse.mybir as mybir
from concourse.bass_utils import run_bass_kernel_spmd

F32 = mybir.dt.float32
F32R = mybir.dt.float32r

B, S, D, H = 4, 2048, 1024, 16
HD = D // H  # 64
N_CORES = 8
HPC = H // N_CORES  # heads per core = 2
J = HPC * HD  # per-core feature dims = 128
BS = B * S  # 8192
NB = S // 128  # 16 s-blocks per batch
NC = S // 512  # 4 chunks of 512 per batch

# fast (relaxed-precision) fp32 for the big matmuls; exact fp32 for transposes.
# fp32r operands must be produced pre-rounded, so every tile feeding an fp32r
# matmul is declared float32r and written by a rounding copy/activation.
MM_DT = F32R


def build_kernel():
    nc = bacc.Bacc(
        "TRN2", target_bir_lowering=False, debug=False, enable_asserts=False,
        num_devices=N_CORES,
    )

    x_d = nc.dram_tensor("x", [BS, D], F32, kind="ExternalInput").ap()
    wq_d = nc.dram_tensor("wq", [D, J], F32, kind="ExternalInput").ap()
    wk_d = nc.dram_tensor("wk", [D, J], F32, kind="ExternalInput").ap()
    wv_d = nc.dram_tensor("wv", [D, J], F32, kind="ExternalInput").ap()
    wo_d = nc.dram_tensor("wo", [J, D], F32, kind="ExternalInput").ap()
    bq_d = nc.dram_tensor("bq", [J], F32, kind="ExternalInput").ap()
    bk_d = nc.dram_tensor("bk", [J], F32, kind="ExternalInput").ap()
    bv_d = nc.dram_tensor("bv", [J], F32, kind="ExternalInput").ap()
    out_d = nc.dram_tensor("out_t", [D, BS], F32, kind="ExternalOutput").ap()

    with tile.TileContext(nc) as tc:
        _emit(tc, nc, x_d, wq_d, wk_d, wv_d, wo_d, bq_d, bk_d, bv_d, out_d)

    nc.compile()
    return nc


def _emit(tc, nc, x_d, wq_d, wk_d, wv_d, wo_d, bq_d, bk_d, bv_d, out_d):
    from contextlib import ExitStack

    ctx = ExitStack()
    with ctx:
        const = ctx.enter_context(tc.tile_pool(name="const", bufs=1))
        wpool = ctx.enter_context(tc.tile_pool(name="w", bufs=1))
        xpool = ctx.enter_context(tc.tile_pool(name="x", bufs=6))
        xtpool = ctx.enter_context(tc.tile_pool(name="xt", bufs=12))
        qkvpool = ctx.enter_context(tc.tile_pool(name="qkv", bufs=2))
        vepool = ctx.enter_context(tc.tile_pool(name="ve", bufs=4))
        ptpool = ctx.enter_context(tc.tile_pool(name="pt", bufs=3))
        aopool = ctx.enter_context(tc.tile_pool(name="ao", bufs=2))
        nrmpool = ctx.enter_context(tc.tile_pool(name="nrm", bufs=2))
        stgpool = ctx.enter_context(tc.tile_pool(name="stg", bufs=3))
        ps_st = ctx.enter_context(tc.tile_pool(name="ps_st", bufs=2, space="PSUM"))
        ps_acc = ctx.enter_context(tc.tile_pool(name="ps_acc", bufs=2, space="PSUM"))
        ps_mm = ctx.enter_context(tc.tile_pool(name="ps_mm", bufs=2, space="PSUM"))

        # --- constants ---------------------------------------------------
        # identity[p, f] = 1 if p == f else 0   (for PE transpose)
        ident = const.tile([128, 128], F32, tag="ident")
        nc.gpsimd.memset(ident[:], 1.0)
        nc.gpsimd.affine_select(
            ident[:], ident[:], pattern=[[1, 128]],
            compare_op=mybir.AluOpType.is_equal, fill=0.0,
            base=0, channel_multiplier=-1,
        )
        # fp32r copy of the identity for transposing fp32r tiles (v^T)
        ident_r = const.tile([128, 128], MM_DT, tag="ident_r")
        nc.vector.tensor_copy(ident_r[:], ident[:])
        # causal mask for diagonal 128x128 blocks of S^T[k, q]:
        # keep (1.0) where k <= q i.e. f - p >= 0
        mask_f = const.tile([128, 128], F32, tag="mask_f")
        nc.gpsimd.memset(mask_f[:], 1.0)
        nc.gpsimd.affine_select(
            mask_f[:], mask_f[:], pattern=[[1, 128]],
            compare_op=mybir.AluOpType.is_ge, fill=0.0,
            base=0, channel_multiplier=-1,
        )
        mask = const.tile([128, 128], MM_DT, tag="mask")
        nc.vector.tensor_copy(mask[:], mask_f[:])
        # fp32r ones column-vector group for the softmax-denominator columns
        ones_f = const.tile([128, 16], F32, tag="ones_f")
        nc.gpsimd.memset(ones_f[:], 1.0)
        ones16 = const.tile([128, 16], MM_DT, tag="ones16")
        nc.vector.tensor_copy(ones16[:], ones_f[:])
        # fp32r ones [128, 64] for the recip partition-broadcast matmul
        ones64f = const.tile([128, 64], F32, tag="ones64f")
        nc.gpsimd.memset(ones64f[:], 1.0)
        ones64 = const.tile([128, 64], MM_DT, tag="ones64")
        nc.vector.tensor_copy(ones64[:], ones64f[:])

        # --- weights -----------------------------------------------------
        # wq/wk/wv: [D, J] -> one [128, 1024] tile per projection (contraction
        # block ib at cols [128*ib, 128*ib+128)). DMA can't cast to fp32r, so
        # stage as fp32 then round with a DVE copy.
        w_tiles = {}
        for name, wd in (("q", wq_d), ("k", wk_d), ("v", wv_d)):
            stg = wpool.tile([128, D], F32, tag="wstg", name="wstg", bufs=2)
            for ib in range(8):
                nc.sync.dma_start(
                    stg[:, ib * 128:(ib + 1) * 128],
                    wd[ib * 128:(ib + 1) * 128, :])
            t = wpool.tile([128, D], MM_DT, tag=f"w{name}", name=f"w{name}")
            nc.vector.tensor_copy(t[:], stg[:])
            w_tiles[name] = t
        wo_stg = wpool.tile([J, D], F32, tag="wstg", name="wo_stg", bufs=2)
        nc.sync.dma_start(wo_stg[:], wo_d[:, :])
        wo_t = wpool.tile([J, D], MM_DT, tag="wo")
        nc.vector.tensor_copy(wo_t[:], wo_stg[:])

        bias = {}
        for name, bd in (("q", bq_d), ("k", bk_d), ("v", bv_d)):
            t = const.tile([J, 1], F32, tag=f"b{name}")
            nc.sync.dma_start(t[:], bd.rearrange("(p o) -> p o", o=1))
            bias[name] = t

        # --- per-batch pipeline -----------------------------------------
        for b in range(B):
            s0 = b * S  # row offset into x / out^T columns

            # projections: q^T/k^T/v^T [J=128, 2048] for this batch.
            # Per 512-wide chunk: load x, PE-transpose to x^T, then the three
            # projection matmuls consume (and release) the chunk's x^T tiles.
            proj = {
                name: qkvpool.tile([J, S], MM_DT, tag=f"{name}t", name=f"{name}t")
                for name in ("q", "k", "v")
            }
            for c in range(NC):
                x_t = []
                for si in range(4):
                    sb = 4 * c + si
                    t = xpool.tile([128, D], F32, tag="x", name="x")
                    nc.sync.dma_start(
                        t[:], x_d[s0 + sb * 128: s0 + (sb + 1) * 128, :])
                    x_t.append(t)
                xt = []
                for ib in range(8):
                    pst = ps_mm.tile([128, 512], F32, tag="ps_mm", name="pst")
                    for si in range(4):
                        nc.tensor.transpose(
                            pst[:, si * 128:(si + 1) * 128],
                            x_t[si][:, ib * 128:(ib + 1) * 128],
                            ident[:],
                        )
                    t = xtpool.tile([128, 512], MM_DT, tag="xt", name="xt")
                    nc.vector.tensor_copy(t[:], pst[:])
                    xt.append(t)
                for name in ("q", "k", "v"):
                    pacc = ps_mm.tile([128, 512], F32, tag="ps_mm", name="pacc")
                    for ib in range(8):
                        nc.tensor.matmul(
                            pacc[:],
                            w_tiles[name][:, ib * 128:(ib + 1) * 128],
                            xt[ib][:],
                            start=(ib == 0), stop=(ib == 7),
                        )
                    # copy PSUM -> SBUF with per-partition bias add (on ACT)
                    nc.scalar.activation(
                        proj[name][:, c * 512:(c + 1) * 512], pacc[:],
                        mybir.ActivationFunctionType.Identity,
                        bias=bias[name][:],
                    )
            qt, kt, vt = proj["q"], proj["k"], proj["v"]

            # V natural (per head, with ones column appended):
            # ve[h]: [128 k, 16*65], block kb at cols [65*kb, 65*kb+65),
            # col 65*kb+64 is the ones column (softmax denominator trick).
            ve = []
            for h in range(HPC):
                t = vepool.tile([128, NB * 65], MM_DT, tag="ve")
                # ones columns at 65*kb + 64 via one strided copy
                nc.vector.tensor_copy(
                    t[:].rearrange("p (nb c) -> p nb c", c=65)[:, :, 64:65],
                    ones16[:].rearrange("p (a o) -> p a o", o=1),
                )
                ve.append(t)
            for sb in range(NB):
                pst = ps_mm.tile([128, 512], F32, tag="ps_mm")
                nc.tensor.transpose(
                    pst[:, 0:128].bitcast(MM_DT),
                    vt[:, sb * 128:(sb + 1) * 128], ident_r[:],
                )
                for h in range(HPC):
                    nc.vector.tensor_copy(
                        ve[h][:, sb * 65: sb * 65 + 64],
                        pst[:, h * 64:(h + 1) * 64].bitcast(MM_DT),
                    )

            # attention for each head
            aot = aopool.tile([J, S], MM_DT, tag="aot")  # attnout^T, heads stacked
            for h in range(HPC):
                hp = slice(h * HD, (h + 1) * HD)  # partition range of this head
                for p in range(2):  # chunk-pair passes: chunks {2p, 2p+1}
                    acc = [
                        ps_acc.tile([128, 512], F32, tag="ps_acc", name="acc0"),
                        ps_acc.tile([128, 512], F32, tag="ps_acc", name="acc1"),
                    ]
                    n_kb = 8 * p + 8
                    for kb in range(n_kb):
                        lo = max(0, 128 * kb - 1024 * p)  # local col offset
                        st = ps_st.tile([128, 1024], F32, tag="ps_st")
                        for half in range(2):
                            hlo = max(lo, 512 * half)
                            hhi = 512 * (half + 1)
                            if hlo >= hhi:
                                continue
                            nc.tensor.matmul(
                                st[:, hlo:hhi],
                                kt[hp, kb * 128:(kb + 1) * 128],
                                qt[hp, 1024 * p + hlo: 1024 * p + hhi],
                                start=True, stop=True,
                            )
                        pt = ptpool.tile([128, 1024], MM_DT, tag="pt")
                        nc.scalar.activation(
                            pt[:, lo:1024], st[:, lo:1024],
                            mybir.ActivationFunctionType.Exp,
                            scale=0.125,
                        )
                        # diagonal block (only when it falls in this pass):
                        # mask the lower triangle
                        if 128 * kb - 1024 * p >= 0:
                            nc.vector.tensor_mul(
                                pt[:, lo:lo + 128], pt[:, lo:lo + 128], mask[:],
                            )
                        for half in range(2):
                            chunk = 2 * p + half
                            if kb > 4 * chunk + 3:
                                continue
                            hlo = max(lo, 512 * half)
                            hhi = 512 * (half + 1)
                            nc.tensor.matmul(
                                acc[half][0:65, hlo - 512 * half: 512],
                                ve[h][:, kb * 65: kb * 65 + 65],
                                pt[:, hlo:hhi],
                                start=(kb == 0), stop=(kb == 4 * chunk + 3),
                            )
                    # normalize: rows 0..63 = unnormalized out^T, row 64 = rowsum
                    for half in range(2):
                        chunk = 2 * p + half
                        rec = nrmpool.tile([128, 512], F32, tag="rec")
                        nc.vector.reciprocal(rec[64:65, :], acc[half][64:65, :])
                        rec_r = nrmpool.tile([128, 512], MM_DT, tag="rec_r")
                        nc.vector.tensor_copy(rec_r[64:65, :], rec[64:65, :])
                        # broadcast recip row to partitions 0..63 via ones-col
                        # matmul (gpsimd partition_broadcast is unreliable)
                        bcp = ps_mm.tile([64, 512], F32, tag="ps_mm", name="bcp")
                        nc.tensor.matmul(
                            bcp[:], ones64[64:65, :], rec_r[64:65, :],
                            start=True, stop=True,
                        )
                        bct = nrmpool.tile([128, 512], F32, tag="bct")
                        nc.vector.tensor_copy(bct[0:64, :], bcp[:])
                        if h == 0:
                            nc.vector.tensor_mul(
                                aot[0:64, chunk * 512:(chunk + 1) * 512],
                                acc[half][0:64, :], bct[0:64, :],
                            )
                        else:
                            tmp = nrmpool.tile([64, 512], MM_DT, tag="tmp")
                            nc.vector.tensor_mul(
                                tmp[:], acc[half][0:64, :], bct[0:64, :],
                            )
                            # partition shift 0-63 -> 64-127 via SBUF->SBUF DMA
                            nc.sync.dma_start(
                                aot[64:128, chunk * 512:(chunk + 1) * 512], tmp[:],
                            )

            # o_proj: out^T[o, s] partial = Wo_slice^T @ attnout^T
            for ob in range(8):
                stg = stgpool.tile([128, S], F32, tag="stg")
                for c in range(NC):
                    pst = ps_mm.tile([128, 512], F32, tag="ps_mm")
                    nc.tensor.matmul(
                        pst[:],
                        wo_t[:, ob * 128:(ob + 1) * 128],
                        aot[:, c * 512:(c + 1) * 512],
                        start=True, stop=True,
                    )
                    nc.vector.tensor_copy(stg[:, c * 512:(c + 1) * 512], pst[:])
                nc.sync.dma_start(
                    out_d[ob * 128:(ob + 1) * 128, s0: s0 + S], stg[:],
                )


_NC_CACHE = None


def _get_nc():
    global _NC_CACHE
    if _NC_CACHE is None:
        _NC_CACHE = build_kernel()
    return _NC_CACHE


def kernel(**inputs) -> np.ndarray:
    x = np.ascontiguousarray(
        np.asarray(inputs["hidden_states"], np.float32).reshape(BS, D))
    Wq = np.asarray(inputs["Wq"], np.float32)
    Wk = np.asarray(inputs["Wk"], np.float32)
    Wv = np.asarray(inputs["Wv"], np.float32)
    Wo = np.asarray(inputs["Wo"], np.float32)
    bq = np.asarray(inputs["bq"], np.float32)
    bk = np.asarray(inputs["bk"], np.float32)
    bv = np.asarray(inputs["bv"], np.float32)
    bo = np.asarray(inputs["bo"], np.float32)

    nc = _get_nc()
    in_maps = []
    for c in range(N_CORES):
        js = slice(c * J, (c + 1) * J)
        in_maps.append({
            "x": x,
            "wq": np.ascontiguousarray(Wq[:, js]),
            "wk": np.ascontiguousarray(Wk[:, js]),
            "wv": np.ascontiguousarray(Wv[:, js]),
            "wo": np.ascontiguousarray(Wo[js, :]),
            "bq": np.ascontiguousarray(bq[js]),
            "bk": np.ascontiguousarray(bk[js]),
            "bv": np.ascontiguousarray(bv[js]),
        })

    res = run_bass_kernel_spmd(nc, in_maps, core_ids=list(range(N_CORES)))
    out_t = np.zeros((D, BS), np.float64)
    for c in range(N_CORES):
        out_t += res.results[c]["out_t"].astype(np.float64)
    out = out_t.T.astype(np.float32) + bo[None, :]
    return out.reshape(B, S, D)


if __name__ == "__main__":
    rng = np.random.default_rng(0)
    ins = {
        "hidden_states": rng.standard_normal((B, S, D), np.float32),
        "Wq": rng.standard_normal((D, D), np.float32) * 0.02,
        "bq": np.zeros(D, np.float32),
        "Wk": rng.standard_normal((D, D), np.float32) * 0.02,
        "bk": np.zeros(D, np.float32),
        "Wv": rng.standard_normal((D, D), np.float32) * 0.02,
        "bv": np.zeros(D, np.float32),
        "Wo": rng.standard_normal((D, D), np.float32) * 0.02,
        "bo": np.zeros(D, np.float32),
    }
    out = kernel(**ins)
    print("out", out.shape, out.dtype, float(np.abs(out).mean()))



# revision 13
# speedup vs baseline: 1.3108x; 1.3108x over previous
"""Trainium2 Bass kernel: GPT-2-style causal multi-head attention (bf16 v2).

Problem: B=4, S=2048, D=1024, H=16 heads (head_dim 64), fp32 reference.
  q/k/v = x @ W{q,k,v} + b{q,k,v}; causal softmax attention; out = attn @ Wo + bo.

Sharding (8 cores): tensor-parallel over heads - each core owns 2 heads
(J=128 feature dims). Wq/Wk/Wv column-sliced, Wo row-sliced per core. Each
core computes a partial o_proj output (transposed, [D, B*S], bf16); the host
sums the 8 partials in fp32, transposes, and adds bo.

v2 changes vs the fp32r baseline (737.8 us):
  * everything bf16 on-chip: halves DMA, halves LDWEIGHTS, full-rate matmul.
  * x is pre-transposed AND pre-cast to bf16 on the host -> no on-chip
    x-transpose (saved 32k PE rows + all the fp32->fp32r CAST copies).
  * w tiles pre-arranged on host into the SBUF lhsT layout.
  * softmax denominators: PE-transpose 8 denom rows into [128, 32], ONE
    reciprocal (free-size 32), transpose back -- replaces the 107 us of
    single-partition [1,512] DVE reciprocals.
  * denominator broadcast via one-hot selector matmul (sel[8,64] lhsT).
  * causal masking via affine_select on the (idle) gpsimd engine.
  * o_proj PSUM->SBUF copies on DVE; q/k/v bias-add copies on DVE
    (tensor_scalar_add); exp stays on ACT: engines balanced.
"""

import sys

sys.path.insert(0, "/opt/trn_rl_repo")

import numpy as np

import concourse.bass as bass
import concourse.bacc as bacc
import concourse.tile as tile
import concourse.mybir as mybir
from concourse.bass_utils import run_bass_kernel_spmd

F32 = mybir.dt.float32
BF16 = mybir.dt.bfloat16

B, S, D, H = 4, 2048, 1024, 16
HD = D // H  # 64
N_CORES = 8
HPC = H // N_CORES  # heads per core = 2
J = HPC * HD  # per-core feature dims = 128
BS = B * S  # 8192
NB = S // 128  # 16 s-blocks per batch
NC = S // 512  # 4 chunks of 512 per batch


def build_kernel():
    nc = bacc.Bacc(
        "TRN2", target_bir_lowering=False, debug=False, enable_asserts=False,
        num_devices=N_CORES,
    )

    xt_d = nc.dram_tensor("xt", [D, BS], BF16, kind="ExternalInput").ap()
    wq_d = nc.dram_tensor("wq", [128, D], BF16, kind="ExternalInput").ap()
    wk_d = nc.dram_tensor("wk", [128, D], BF16, kind="ExternalInput").ap()
    wv_d = nc.dram_tensor("wv", [128, D], BF16, kind="ExternalInput").ap()
    wo_d = nc.dram_tensor("wo", [J, D], BF16, kind="ExternalInput").ap()
    bq_d = nc.dram_tensor("bq", [J], F32, kind="ExternalInput").ap()
    bk_d = nc.dram_tensor("bk", [J], F32, kind="ExternalInput").ap()
    bv_d = nc.dram_tensor("bv", [J], F32, kind="ExternalInput").ap()
    # host-precomputed constants (walrus rejects some affine_select forms)
    cid_d = nc.dram_tensor("cident", [128, 128], BF16, kind="ExternalInput").ap()
    cmask_d = nc.dram_tensor("cmask", [128, 128], BF16, kind="ExternalInput").ap()
    cones_d = nc.dram_tensor("cones", [128, 16], BF16, kind="ExternalInput").ap()
    csel_d = nc.dram_tensor("csel", [8, 512], BF16, kind="ExternalInput").ap()
    out_d = nc.dram_tensor("out_t", [D, BS], BF16, kind="ExternalOutput").ap()

    with tile.TileContext(nc) as tc:
        _emit(tc, nc, xt_d, wq_d, wk_d, wv_d, wo_d, bq_d, bk_d, bv_d,
              cid_d, cmask_d, cones_d, csel_d, out_d)

    nc.compile()
    return nc


def _emit(tc, nc, xt_d, wq_d, wk_d, wv_d, wo_d, bq_d, bk_d, bv_d,
          cid_d, cmask_d, cones_d, csel_d, out_d):
    from contextlib import ExitStack

    ctx = ExitStack()
    with ctx:
        const = ctx.enter_context(tc.tile_pool(name="const", bufs=1))
        wpool = ctx.enter_context(tc.tile_pool(name="w", bufs=1))
        xpool = ctx.enter_context(tc.tile_pool(name="x", bufs=12))
        qkvpool = ctx.enter_context(tc.tile_pool(name="qkv", bufs=2))
        vepool = ctx.enter_context(tc.tile_pool(name="ve", bufs=2))
        ptpool = ctx.enter_context(tc.tile_pool(name="pt", bufs=8))
        aoupool = ctx.enter_context(tc.tile_pool(name="aou", bufs=12))
        aotpool = ctx.enter_context(tc.tile_pool(name="aot", bufs=2))
        nrmpool = ctx.enter_context(tc.tile_pool(name="nrm", bufs=2))
        stgpool = ctx.enter_context(tc.tile_pool(name="stg", bufs=3))
        # PSUM: pool A (proj / transposes / o_proj / bcast) 2 banks,
        # pool B (scores) 3 banks, pool C (PV accum) 3 banks -> 8 total
        ps_a = ctx.enter_context(tc.tile_pool(name="ps_a", bufs=2, space="PSUM"))
        ps_b = ctx.enter_context(tc.tile_pool(name="ps_b", bufs=3, space="PSUM"))
        ps_c = ctx.enter_context(tc.tile_pool(name="ps_c", bufs=3, space="PSUM"))

        # --- constants (host-precomputed, DMA'd in) ----------------------
        ident_b = const.tile([128, 128], BF16, tag="ident_b")
        nc.sync.dma_start(ident_b[:], cid_d[:, :])
        mask_b = const.tile([128, 128], BF16, tag="mask_b")
        nc.sync.dma_start(mask_b[:], cmask_d[:, :])
        ones16 = const.tile([128, 16], BF16, tag="ones16")
        nc.sync.dma_start(ones16[:], cones_d[:, :])
        # selector for the denominator broadcast: sel[:, r*64:(r+1)*64] is
        # [8, 64] with row r all-ones -> matmul(out[64,512], sel_r, rdn)
        # broadcasts rdn row r across 64 partitions.
        sel = const.tile([8, 8 * 64], BF16, tag="sel")
        nc.sync.dma_start(sel[:], csel_d[:, :])

        # --- weights -----------------------------------------------------
        w_tiles = {}
        for name, wd in (("q", wq_d), ("k", wk_d), ("v", wv_d)):
            t = wpool.tile([128, D], BF16, tag=f"w{name}")
            nc.sync.dma_start(t[:], wd[:, :])
            w_tiles[name] = t
        wo_t = wpool.tile([J, D], BF16, tag="wo")
        nc.sync.dma_start(wo_t[:], wo_d[:, :])

        bias = {}
        for name, bd in (("q", bq_d), ("k", bk_d), ("v", bv_d)):
            t = const.tile([J, 1], F32, tag=f"b{name}")
            nc.sync.dma_start(t[:], bd.rearrange("(p o) -> p o", o=1))
            bias[name] = t

        # --- per-batch pipeline -----------------------------------------
        for b in range(B):
            s0 = b * S

            # x^T tiles for this batch: 8 x [128, 2048] bf16
            xt = []
            for ib in range(8):
                t = xpool.tile([128, S], BF16, tag="xt", name="xt")
                nc.sync.dma_start(
                    t[:], xt_d[ib * 128:(ib + 1) * 128, s0: s0 + S])
                xt.append(t)

            # projections q^T/k^T/v^T: [J=128, 2048] bf16
            proj = {}
            for name in ("q", "k", "v"):
                pt_ = qkvpool.tile([J, S], BF16, tag=f"{name}t", name=f"{name}t")
                proj[name] = pt_
                for c in range(NC):
                    pacc = ps_a.tile([128, 512], F32, tag="ps_a", name="pacc")
                    for ib in range(8):
                        nc.tensor.matmul(
                            pacc[:],
                            w_tiles[name][:, ib * 128:(ib + 1) * 128],
                            xt[ib][:, c * 512:(c + 1) * 512],
                            start=(ib == 0), stop=(ib == 7),
                        )
                    # PSUM -> SBUF bf16 with per-partition bias add on DVE
                    nc.vector.tensor_scalar_add(
                        pt_[:, c * 512:(c + 1) * 512], pacc[:], bias[name][:],
                    )
            qt, kt, vt = proj["q"], proj["k"], proj["v"]

            # V natural (per head, with ones column appended):
            # ve[h]: [128 k, 16*65] bf16, block kb at cols [65*kb, 65*kb+65),
            # col 65*kb+64 is the ones column (softmax denominator trick).
            ve = []
            for h in range(HPC):
                t = vepool.tile([128, NB * 65], BF16, tag=f"ve{h}")
                nc.vector.tensor_copy(
                    t[:].rearrange("p (nb c) -> p nb c", c=65)[:, :, 64:65],
                    ones16[:].rearrange("p (a o) -> p a o", o=1),
                )
                ve.append(t)
            for sb in range(NB):
                pst = ps_a.tile([128, 128], BF16, tag="ps_a", name="vtp")
                nc.tensor.transpose(
                    pst[:], vt[:, sb * 128:(sb + 1) * 128], ident_b[:],
                )
                for h in range(HPC):
                    nc.vector.tensor_copy(
                        ve[h][:, sb * 65: sb * 65 + 64],
                        pst[:, h * 64:(h + 1) * 64],
                    )

            # --- attention ---------------------------------------------
            # dn[8, 512] bf16: row 4*h+c holds the softmax denominators of
            # head h, chunk c (q-cols local to the chunk).
            dn = nrmpool.tile([8, 512], BF16, tag="dn")
            aou = {}  # unnormalized attnout^T [64, 512] bf16 per (h, c)
            for c in range(NC):
                for h in range(HPC):
                    hp = slice(h * HD, (h + 1) * HD)
                    acc = ps_c.tile([128, 512], F32, tag="ps_c", name="acc")
                    n_kb = 4 * c + 4
                    for kb in range(n_kb):
                        lo = max(0, 128 * kb - 512 * c)
                        st = ps_b.tile([128, 512], F32, tag="ps_b", name="st")
                        nc.tensor.matmul(
                            st[:, lo:512],
                            kt[hp, kb * 128:(kb + 1) * 128],
                            qt[hp, 512 * c + lo: 512 * (c + 1)],
                            start=True, stop=True,
                        )
                        pt = ptpool.tile([128, 512], BF16, tag="pt", name="pt")
                        nc.scalar.activation(
                            pt[:, lo:512], st[:, lo:512],
                            mybir.ActivationFunctionType.Exp,
                            scale=0.125,
                        )
                        if kb >= 4 * c:  # diagonal block: causal mask
                            nc.gpsimd.tensor_mul(
                                pt[:, lo:lo + 128], pt[:, lo:lo + 128],
                                mask_b[:],
                            )
                        nc.tensor.matmul(
                            acc[0:65, lo:512],
                            ve[h][:, kb * 65: kb * 65 + 65],
                            pt[:, lo:512],
                            start=(kb == 0), stop=(kb == n_kb - 1),
                        )
                    # stash unnormalized out^T + denom row
                    t = aoupool.tile([64, 512], BF16, tag="aou", name="aou")
                    nc.scalar.activation(
                        t[:], acc[0:64, :],
                        mybir.ActivationFunctionType.Identity,
                    )
                    aou[(h, c)] = t
                    # denom row lives on PSUM partition 64; engines can't
                    # shift partitions, so copy to SBUF p64 then DMA-shift
                    dstg = aoupool.tile([128, 512], BF16, tag="dstg",
                                        name="dstg", bufs=3)
                    nc.vector.tensor_copy(dstg[64:65, :], acc[64:65, :])
                    r = 4 * h + c
                    nc.sync.dma_start(dn[r:r + 1, :], dstg[64:65, :])

            # one reciprocal for the whole batch: transpose dn -> [128, 32],
            # recip, transpose back -> rdn [8, 512] bf16
            dnt_ps = ps_a.tile([128, 32], BF16, tag="ps_a", name="dnt")
            for jblk in range(4):
                nc.tensor.transpose(
                    dnt_ps[:, jblk * 8:(jblk + 1) * 8],
                    dn[0:8, jblk * 128:(jblk + 1) * 128],
                    ident_b[0:8, 0:8],
                )
            dnt = nrmpool.tile([128, 32], F32, tag="dnt")
            nc.vector.tensor_copy(dnt[:], dnt_ps[:])
            rdnt = nrmpool.tile([128, 32], BF16, tag="rdnt")
            with nc.allow_low_precision(reason="bf16 softmax denominators"):
                nc.vector.reciprocal(rdnt[:], dnt[:])
            rdn_ps = ps_a.tile([8, 512], BF16, tag="ps_a", name="rdnps")
            for jblk in range(4):
                nc.tensor.transpose(
                    rdn_ps[0:8, jblk * 128:(jblk + 1) * 128],
                    rdnt[:, jblk * 8:(jblk + 1) * 8],
                    ident_b[:],
                )
            rdn = nrmpool.tile([8, 512], BF16, tag="rdn")
            nc.vector.tensor_copy(rdn[:], rdn_ps[:])

            # normalize + assemble attnout^T [128, 2048] bf16
            aot = aotpool.tile([J, S], BF16, tag="aot")
            for c in range(NC):
                for h in range(HPC):
                    r = 4 * h + c
                    bcp = ps_a.tile([64, 512], F32, tag="ps_a", name="bcp")
                    nc.tensor.matmul(
                        bcp[:], sel[:, r * 64:(r + 1) * 64], rdn[:],
                        start=True, stop=True,
                    )
                    if h == 0:
                        nc.vector.tensor_mul(
                            aot[0:64, c * 512:(c + 1) * 512],
                            aou[(h, c)][:], bcp[:],
                        )
                    else:
                        tmp = nrmpool.tile([64, 512], BF16, tag="tmp",
                                           name="tmp", bufs=3)
                        nc.vector.tensor_mul(tmp[:], aou[(h, c)][:], bcp[:])
                        # partition shift 0-63 -> 64-127 via SBUF->SBUF DMA
                        nc.sync.dma_start(
                            aot[64:128, c * 512:(c + 1) * 512], tmp[:],
                        )

            # o_proj: out^T[o, s] partial = Wo_slice^T @ attnout^T
            for ob in range(8):
                stg = stgpool.tile([128, S], BF16, tag="stg")
                for c in range(NC):
                    pst = ps_a.tile([128, 512], F32, tag="ps_a", name="pst")
                    nc.tensor.matmul(
                        pst[:],
                        wo_t[:, ob * 128:(ob + 1) * 128],
                        aot[:, c * 512:(c + 1) * 512],
                        start=True, stop=True,
                    )
                    nc.vector.tensor_copy(stg[:, c * 512:(c + 1) * 512], pst[:])
                nc.sync.dma_start(
                    out_d[ob * 128:(ob + 1) * 128, s0: s0 + S], stg[:],
                )


_NC_CACHE = None


def _get_nc():
    global _NC_CACHE
    if _NC_CACHE is None:
        _NC_CACHE = build_kernel()
    return _NC_CACHE


def make_in_maps(inputs):
    """Host-side prep: cast to bf16, pre-transpose x, pre-tile weights."""
    import ml_dtypes
    bf = ml_dtypes.bfloat16
    x = np.asarray(inputs["hidden_states"], np.float32).reshape(BS, D)
    xt = np.ascontiguousarray(x.T.astype(bf))  # [D, BS] bf16
    Wq = np.asarray(inputs["Wq"], np.float32)
    Wk = np.asarray(inputs["Wk"], np.float32)
    Wv = np.asarray(inputs["Wv"], np.float32)
    Wo = np.asarray(inputs["Wo"], np.float32)
    bq = np.asarray(inputs["bq"], np.float32)
    bk = np.asarray(inputs["bk"], np.float32)
    bv = np.asarray(inputs["bv"], np.float32)

    def wtile(W, js):
        # [D, 128] column slice -> lhsT tile layout [128, 1024]:
        # tile[p, ib*128 + j] = W[ib*128 + p, js.start + j]
        return np.ascontiguousarray(
            W[:, js].reshape(8, 128, 128).transpose(1, 0, 2).reshape(128, D)
            .astype(bf))

    cident = np.eye(128, dtype=bf)
    cmask = np.tril(np.ones((128, 128), np.float32)).T.astype(bf)
    cones = np.ones((128, 16), bf)
    csel = np.zeros((8, 512), np.float32)
    for r in range(8):
        csel[r, r * 64:(r + 1) * 64] = 1.0
    csel = csel.astype(bf)

    in_maps = []
    for c in range(N_CORES):
        js = slice(c * J, (c + 1) * J)
        in_maps.append({
            "xt": xt,
            "wq": wtile(Wq, js),
            "wk": wtile(Wk, js),
            "wv": wtile(Wv, js),
            "wo": np.ascontiguousarray(Wo[js, :].astype(bf)),
            "bq": np.ascontiguousarray(bq[js]),
            "bk": np.ascontiguousarray(bk[js]),
            "bv": np.ascontiguousarray(bv[js]),
            "cident": cident,
            "cmask": cmask,
            "cones": cones,
            "csel": csel,
        })
    return in_maps


def gather_output(results, bo):
    out_t = np.zeros((D, BS), np.float32)
    for c in range(N_CORES):
        out_t += results[c]["out_t"].astype(np.float32)
    out = out_t.T + np.asarray(bo, np.float32)[None, :]
    return out.reshape(B, S, D)


def kernel(**inputs) -> np.ndarray:
    nc = _get_nc()
    in_maps = make_in_maps(inputs)
    res = run_bass_kernel_spmd(nc, in_maps, core_ids=list(range(N_CORES)))
    return gather_output(res.results, inputs["bo"])


if __name__ == "__main__":
    rng = np.random.default_rng(0)
    ins = {
        "hidden_states": rng.standard_normal((B, S, D), np.float32),
        "Wq": rng.standard_normal((D, D), np.float32) * 0.02,
        "bq": np.zeros(D, np.float32),
        "Wk": rng.standard_normal((D, D), np.float32) * 0.02,
        "bk": np.zeros(D, np.float32),
        "Wv": rng.standard_normal((D, D), np.float32) * 0.02,
        "bv": np.zeros(D, np.float32),
        "Wo": rng.standard_normal((D, D), np.float32) * 0.02,
        "bo": np.zeros(D, np.float32),
    }
    out = kernel(**ins)
    print("out", out.shape, out.dtype, float(np.abs(out).mean()))


# revision 26
# speedup vs baseline: 1.7478x; 1.3334x over previous
"""Trainium2 Bass kernel: GPT-2-style causal multi-head attention (bf16 v3).

Problem: B=4, S=2048, D=1024, H=16 heads (head_dim 64), fp32 reference.
  q/k/v = x @ W{q,k,v} + b{q,k,v}; causal softmax attention; out = attn @ Wo + bo.

Sharding (8 cores): tensor-parallel over heads - each core owns 2 heads
(J=128 feature dims). Wq/Wk/Wv column-sliced, Wo row-sliced per core. Each
core computes a partial o_proj output (transposed, [D, B*S], bf16); the host
sums the 8 partials in fp32, transposes, and adds bo.

v3 (from v2, 562.9 us): the attention inner loop was dependency-stalled
(scores -> exp[ACT] -> mask[gpsimd] -> PV chain made the in-order PE wait
~500 ns/iter and kept it at mid p-state). Fixes:
  * software pipelining: scores emitted LA=3 iterations ahead of PV.
  * causal mask off the critical path: PV of a diagonal tile is split into
    an unmasked part (issues right after exp) + the masked 128 cols.
  * fine-grained interleave: QKV matmuls of batch b+1 and o_proj matmuls of
    batch b-1 are emitted as filler between attention iterations of batch b,
    so the PE never idles (and stays at full p-state).
  * ve copies batched into single strided 4D-AP copies (was 128 tiny CASTs).
  * plain tensor_copy for q/k/v PSUM->SBUF when biases are all zero
    (tensor_scalar_add costs ~3x a copy on DVE); bias variant kept.
"""

import sys

sys.path.insert(0, "/opt/trn_rl_repo")

import numpy as np

import concourse.bass as bass
import concourse.bacc as bacc
import concourse.tile as tile
import concourse.mybir as mybir
from concourse.bass_utils import run_bass_kernel_spmd

F32 = mybir.dt.float32
BF16 = mybir.dt.bfloat16

B, S, D, H = 4, 2048, 1024, 16
HD = D // H  # 64
N_CORES = 8
HPC = H // N_CORES  # heads per core = 2
J = HPC * HD  # per-core feature dims = 128
BS = B * S  # 8192
NB = S // 128  # 16 s-blocks per batch
NC = S // 512  # 4 chunks of 512 per batch
LA = 3  # scores lookahead (must be <= ps_b bufs)


class FillQueue:
    """Queue of single-instruction emitters with chain boundaries.

    Items: ("u", fn) plain unit, ("b", fn) chain begin, ("e", fn) chain end.
    pop(n) emits n units; drain_chain() finishes an open chain so PSUM ring
    slots held by a partially-emitted accumulation chain get released before
    an out-of-band allocation (avoids tile-scheduler deadlock).
    """

    def __init__(self):
        self.items = []
        self.pos = 0
        self.in_chain = False

    def push(self, kind, fn):
        self.items.append((kind, fn))

    def _step(self):
        kind, fn = self.items[self.pos]
        self.pos += 1
        fn()
        if kind == "b":
            self.in_chain = True
        elif kind == "e":
            self.in_chain = False

    def pop(self, n):
        for _ in range(n):
            if self.pos >= len(self.items):
                return
            self._step()

    def drain_chain(self):
        while self.in_chain and self.pos < len(self.items):
            self._step()

    def drain_all(self):
        while self.pos < len(self.items):
            self._step()


def build_kernel(with_bias=False, n_batches=B, debug_outs=()):
    nc = bacc.Bacc(
        "TRN2", target_bir_lowering=False, debug=False, enable_asserts=False,
        num_devices=N_CORES,
    )

    t = {}
    t["xt"] = nc.dram_tensor("xt", [D, BS], BF16, kind="ExternalInput").ap()
    for w in ("wq", "wk", "wv"):
        t[w] = nc.dram_tensor(w, [128, D], BF16, kind="ExternalInput").ap()
    t["wo"] = nc.dram_tensor("wo", [J, D], BF16, kind="ExternalInput").ap()
    for bn in ("bq", "bk", "bv"):
        t[bn] = nc.dram_tensor(bn, [J], F32, kind="ExternalInput").ap()
    t["cident"] = nc.dram_tensor("cident", [128, 128], BF16,
                                 kind="ExternalInput").ap()
    t["cmask"] = nc.dram_tensor("cmask", [128, 128], BF16,
                                kind="ExternalInput").ap()
    t["cones"] = nc.dram_tensor("cones", [128, 32], BF16,
                                kind="ExternalInput").ap()
    t["csel"] = nc.dram_tensor("csel", [8, 512], BF16,
                               kind="ExternalInput").ap()
    t["out"] = nc.dram_tensor("out_t", [D, BS], BF16, kind="ExternalOutput").ap()
    for dbg in debug_outs:
        t[f"dbg_{dbg}"] = nc.dram_tensor(
            f"dbg_{dbg}", [128, 4096], BF16, kind="ExternalOutput").ap()

    with tile.TileContext(nc) as tc:
        _emit(tc, nc, t, with_bias, n_batches, debug_outs)

    nc.compile()
    return nc


def _emit(tc, nc, td, with_bias, n_batches=B, debug_outs=()):
    from contextlib import ExitStack

    ctx = ExitStack()
    with ctx:
        const = ctx.enter_context(tc.tile_pool(name="const", bufs=1))
        wpool = ctx.enter_context(tc.tile_pool(name="w", bufs=1))
        xpool = ctx.enter_context(tc.tile_pool(name="x", bufs=12))
        qkvpool = ctx.enter_context(tc.tile_pool(name="qkv", bufs=2))
        vepool = ctx.enter_context(tc.tile_pool(name="ve", bufs=2))
        ptpool = ctx.enter_context(tc.tile_pool(name="pt", bufs=8))
        aoupool = ctx.enter_context(tc.tile_pool(name="aou", bufs=12))
        aotpool = ctx.enter_context(tc.tile_pool(name="aot", bufs=2))
        nrmpool = ctx.enter_context(tc.tile_pool(name="nrm", bufs=2))
        stgpool = ctx.enter_context(tc.tile_pool(name="stg", bufs=3))
        # PSUM banks: ps_a 2 + ps_b 3 + ps_c 3 = 8
        ps_a = ctx.enter_context(tc.tile_pool(name="ps_a", bufs=2, space="PSUM"))
        ps_b = ctx.enter_context(tc.tile_pool(name="ps_b", bufs=3, space="PSUM"))
        ps_c = ctx.enter_context(tc.tile_pool(name="ps_c", bufs=3, space="PSUM"))

        # --- constants (host-precomputed) --------------------------------
        ident_b = const.tile([128, 128], BF16, tag="ident_b")
        nc.sync.dma_start(ident_b[:], td["cident"][:, :])
        mask_b = const.tile([128, 128], BF16, tag="mask_b")
        nc.sync.dma_start(mask_b[:], td["cmask"][:, :])
        ones32 = const.tile([128, 32], BF16, tag="ones32")
        nc.sync.dma_start(ones32[:], td["cones"][:, :])
        sel = const.tile([8, 512], BF16, tag="sel")
        nc.sync.dma_start(sel[:], td["csel"][:, :])

        # --- weights -----------------------------------------------------
        w_tiles = {}
        for name, key in (("q", "wq"), ("k", "wk"), ("v", "wv")):
            wt = wpool.tile([128, D], BF16, tag=f"w{name}")
            nc.sync.dma_start(wt[:], td[key][:, :])
            w_tiles[name] = wt
        wo_t = wpool.tile([J, D], BF16, tag="wo")
        nc.sync.dma_start(wo_t[:], td["wo"][:, :])

        bias = {}
        for name, key in (("q", "bq"), ("k", "bk"), ("v", "bv")):
            bt = const.tile([J, 1], F32, tag=f"b{name}")
            nc.sync.dma_start(bt[:], td[key].rearrange("(p o) -> p o", o=1))
            bias[name] = bt

        # --- per-batch building blocks -----------------------------------

        def load_xt(b):
            xt = []
            for ib in range(8):
                xti = xpool.tile([128, S], BF16, tag="xt", name="xt")
                nc.sync.dma_start(
                    xti[:], td["xt"][ib * 128:(ib + 1) * 128,
                                     b * S: b * S + S])
                xt.append(xti)
            return xt

        def qkv_jobs(fq, b, xt, proj_out):
            """Push the 12 projection chain jobs for batch b into fq.
            Tiles are allocated lazily inside the closures (pool.tile() is a
            program-order event; eager allocation would deadlock the rings).
            """
            hold = {}

            def mk_proj_alloc(nm):
                def f():
                    proj_out[nm] = qkvpool.tile(
                        [J, S], BF16, tag=f"{nm}t", name=f"{nm}t")
                return f

            def mk_mm(nm, ib_, c_):
                def f():
                    if ib_ == 0:
                        hold["pacc"] = ps_a.tile(
                            [128, 512], F32, tag="ps_a", name="pacc")
                    nc.tensor.matmul(
                        hold["pacc"][:],
                        w_tiles[nm][:, ib_ * 128:(ib_ + 1) * 128],
                        xt[ib_][:, c_ * 512:(c_ + 1) * 512],
                        start=(ib_ == 0), stop=(ib_ == 7),
                    )
                return f

            def mk_cp(nm, c_):
                def f():
                    dst = proj_out[nm][:, c_ * 512:(c_ + 1) * 512]
                    if with_bias:
                        nc.vector.tensor_scalar_add(
                            dst, hold["pacc"][:], bias[nm][:])
                    else:
                        nc.vector.tensor_copy(dst, hold["pacc"][:])
                return f

            for name in ("q", "k", "v"):
                fq.push("u", mk_proj_alloc(name))
                for c in range(NC):
                    for ib in range(8):
                        fq.push("b" if ib == 0 else "u", mk_mm(name, ib, c))
                    fq.push("e", mk_cp(name, c))

        def ve_jobs(fq, b, proj_src, ve_out):
            """V natural [k, hd] with ones cols: ve2 layout per kb block of
            130 cols: [v_h0(64) | one | v_h1(64) | one]. Lazy tile allocs."""
            hold = {}

            def ones_cp():
                ve2 = vepool.tile([128, NB * 130], BF16, tag="ve2")
                ve_out.append(ve2)
                view = ve2[:].rearrange("p (kb h c) -> p kb h c", h=2, c=65)
                nc.vector.tensor_copy(
                    view[:, :, :, 64:65],
                    ones32[:].rearrange("p (kb h o) -> p kb h o", h=2, o=1),
                )
            fq.push("u", ones_cp)
            for g in range(4):  # groups of 4 s-blocks

                def mk_tr(g_, i_):
                    def f():
                        if i_ == 0:
                            hold["pst"] = ps_a.tile(
                                [128, 512], BF16, tag="ps_a", name="vtp")
                        sb = g_ * 4 + i_
                        nc.tensor.transpose(
                            hold["pst"][:, i_ * 128:(i_ + 1) * 128],
                            proj_src["v"][:, sb * 128:(sb + 1) * 128],
                            ident_b[:],
                        )
                    return f

                def mk_cp(g_):
                    def f():
                        view = ve_out[0][:].rearrange(
                            "p (kb h c) -> p kb h c", h=2, c=65)
                        nc.vector.tensor_copy(
                            view[:, g_ * 4:(g_ + 1) * 4, :, 0:64],
                            hold["pst"][:].rearrange(
                                "p (s hh cc) -> p s hh cc", s=4, cc=64),
                        )
                    return f

                for i in range(4):
                    fq.push("b" if i == 0 else "u", mk_tr(g, i))
                fq.push("e", mk_cp(g))

        def oproj_jobs(fq, b, aot_src):
            hold = {}
            for ob in range(8):
                for c in range(NC):

                    def mk_mm(ob_, c_):
                        def f():
                            if c_ == 0:
                                hold["stg"] = stgpool.tile(
                                    [128, S], BF16, tag="stg", name="stg")
                            hold["pst"] = ps_a.tile(
                                [128, 512], F32, tag="ps_a", name="pst")
                            nc.tensor.matmul(
                                hold["pst"][:],
                                wo_t[:, ob_ * 128:(ob_ + 1) * 128],
                                aot_src[0][:, c_ * 512:(c_ + 1) * 512],
                                start=True, stop=True,
                            )
                        return f

                    def mk_cp(c_):
                        def f():
                            nc.vector.tensor_copy(
                                hold["stg"][:, c_ * 512:(c_ + 1) * 512],
                                hold["pst"][:])
                        return f

                    fq.push("b", mk_mm(ob, c))
                    fq.push("e", mk_cp(c))

                def mk_dma(ob_):
                    def f():
                        nc.sync.dma_start(
                            td["out"][ob_ * 128:(ob_ + 1) * 128,
                                      b * S: b * S + S], hold["stg"][:])
                    return f
                fq.push("u", mk_dma(ob))

        def attention(b, qt, kt, vt, ve2, fq):
            """The latency-critical part, run inline with filler pops."""
            dn = nrmpool.tile([8, 512], BF16, tag="dn")
            aou = {}
            for c in range(NC):
                for h in range(HPC):
                    hp = slice(h * HD, (h + 1) * HD)
                    acc = ps_c.tile([128, 512], F32, tag="ps_c", name="acc")
                    n_kb = 4 * c + 4
                    pts = {}

                    def emit_scores(kb):
                        lo = max(0, 128 * kb - 512 * c)
                        st = ps_b.tile([128, 512], F32, tag="ps_b", name="st")
                        nc.tensor.matmul(
                            st[:, lo:512],
                            kt[hp, kb * 128:(kb + 1) * 128],
                            qt[hp, 512 * c + lo: 512 * (c + 1)],
                            start=True, stop=True,
                        )
                        pt = ptpool.tile([128, 512], BF16, tag="pt", name="pt")
                        nc.scalar.activation(
                            pt[:, lo:512], st[:, lo:512],
                            mybir.ActivationFunctionType.Exp,
                            scale=0.125,
                        )
                        if kb >= 4 * c:  # diagonal: mask (off critical path)
                            nc.gpsimd.tensor_mul(
                                pt[:, lo:lo + 128], pt[:, lo:lo + 128],
                                mask_b[:],
                            )
                        pts[kb] = pt

                    def emit_pv(kb):
                        lo = max(0, 128 * kb - 512 * c)
                        pt = pts.pop(kb)
                        last = kb == n_kb - 1
                        vsl = ve2[:, kb * 130 + 65 * h: kb * 130 + 65 * h + 65]
                        if kb >= 4 * c and lo + 128 < 512:
                            # unmasked columns first (independent of mask).
                            # start=True zero-marks the WHOLE psum bank, so
                            # exactly one start per accumulation chain.
                            nc.tensor.matmul(
                                acc[0:65, lo + 128:512], vsl,
                                pt[:, lo + 128:512],
                                start=(kb == 0), stop=False,
                            )
                            nc.tensor.matmul(
                                acc[0:65, lo:lo + 128], vsl,
                                pt[:, lo:lo + 128],
                                start=False, stop=last,
                            )
                        else:
                            nc.tensor.matmul(
                                acc[0:65, lo:512], vsl, pt[:, lo:512],
                                start=(kb == 0), stop=last,
                            )

                    for kb in range(min(LA, n_kb)):
                        emit_scores(kb)
                    for kb in range(n_kb):
                        if kb + LA < n_kb:
                            emit_scores(kb + LA)
                        emit_pv(kb)
                        fq.pop(2)

                    # stash unnormalized out^T (DVE) + denom row
                    t_ = aoupool.tile([64, 512], BF16, tag="aou", name="aou")
                    nc.vector.tensor_copy(t_[:], acc[0:64, :])
                    aou[(h, c)] = t_
                    dstg = aoupool.tile([128, 512], BF16, tag="dstg",
                                        name="dstg", bufs=3)
                    nc.vector.tensor_copy(dstg[64:65, :], acc[64:65, :])
                    r = 4 * h + c
                    nc.sync.dma_start(dn[r:r + 1, :], dstg[64:65, :])
                    fq.pop(2)
            return dn, aou

        def normalize(b, dn, aou, fq):
            """One reciprocal for the whole batch via transpose dance."""
            fq.drain_chain()
            dnt_ps = ps_a.tile([128, 32], BF16, tag="ps_a", name="dnt")
            for jblk in range(4):
                nc.tensor.transpose(
                    dnt_ps[:, jblk * 8:(jblk + 1) * 8],
                    dn[0:8, jblk * 128:(jblk + 1) * 128],
                    ident_b[0:8, 0:8],
                )
            dnt = nrmpool.tile([128, 32], F32, tag="dnt")
            nc.vector.tensor_copy(dnt[:], dnt_ps[:])
            rdnt = nrmpool.tile([128, 32], BF16, tag="rdnt")
            with nc.allow_low_precision(reason="bf16 softmax denominators"):
                nc.vector.reciprocal(rdnt[:], dnt[:])
            fq.drain_chain()
            rdn_ps = ps_a.tile([8, 512], BF16, tag="ps_a", name="rdnps")
            for jblk in range(4):
                nc.tensor.transpose(
                    rdn_ps[0:8, jblk * 128:(jblk + 1) * 128],
                    rdnt[:, jblk * 8:(jblk + 1) * 8],
                    ident_b[:],
                )
            rdn = nrmpool.tile([8, 512], BF16, tag="rdn")
            nc.vector.tensor_copy(rdn[:], rdn_ps[:])

            aot = aotpool.tile([J, S], BF16, tag="aot")
            for c in range(NC):
                for h in range(HPC):
                    r = 4 * h + c
                    fq.drain_chain()
                    bcp = ps_a.tile([64, 512], F32, tag="ps_a", name="bcp")
                    nc.tensor.matmul(
                        bcp[:], sel[:, r * 64:(r + 1) * 64], rdn[:],
                        start=True, stop=True,
                    )
                    if h == 0:
                        nc.vector.tensor_mul(
                            aot[0:64, c * 512:(c + 1) * 512],
                            aou[(h, c)][:], bcp[:],
                        )
                    else:
                        tmp = nrmpool.tile([64, 512], BF16, tag="tmp",
                                           name="tmp", bufs=3)
                        nc.vector.tensor_mul(tmp[:], aou[(h, c)][:], bcp[:])
                        nc.sync.dma_start(
                            aot[64:128, c * 512:(c + 1) * 512], tmp[:],
                        )
                    fq.pop(2)
            return aot

        # --- software-pipelined batch schedule ---------------------------
        # batch b attention interleaves: o_proj of b-1, then qkv+ve of b+1.
        xt = {0: load_xt(0)}
        proj = {}
        ve = {}
        aot = {}

        # prologue: batch 0 projections emitted directly
        fq0 = FillQueue()
        proj[0] = {}
        qkv_jobs(fq0, 0, xt[0], proj[0])
        ve[0] = []
        ve_jobs(fq0, 0, proj[0], ve[0])
        fq0.drain_all()

        def dump(name, src):
            if name in debug_outs:
                p, w = src.shape[0], src.shape[-1]
                nc.sync.dma_start(td[f"dbg_{name}"][0:p, 0:w], src)

        NBATCH = n_batches
        for b in range(NBATCH):
            fq = FillQueue()
            if b - 1 in aot:
                oproj_jobs(fq, b - 1, [aot[b - 1]])
            if b + 1 < NBATCH:
                xt[b + 1] = load_xt(b + 1)
                proj[b + 1] = {}
                qkv_jobs(fq, b + 1, xt[b + 1], proj[b + 1])
                ve[b + 1] = []
                ve_jobs(fq, b + 1, proj[b + 1], ve[b + 1])
                xt.pop(b, None)

            if b == 0:
                for nm in ("q", "k", "v"):
                    dump(nm + "t", proj[0][nm][:])
                dump("ve2", ve[0][0][:])

            dn, aou = attention(
                b, proj[b]["q"], proj[b]["k"], proj[b]["v"], ve[b][0], fq)
            aot[b] = normalize(b, dn, aou, fq)
            if b == 0:
                dump("aot", aot[0][:])
                dump("dn", dn[:])
            fq.drain_all()
            proj.pop(b, None)
            ve.pop(b, None)

        # epilogue: o_proj of the last batch
        fqz = FillQueue()
        oproj_jobs(fqz, NBATCH - 1, [aot[NBATCH - 1]])
        fqz.drain_all()


_NC_CACHE = {}


def _get_nc(with_bias=False):
    if with_bias not in _NC_CACHE:
        _NC_CACHE[with_bias] = build_kernel(with_bias)
    return _NC_CACHE[with_bias]


def make_in_maps(inputs):
    """Host-side prep: cast to bf16, pre-transpose x, pre-tile weights."""
    import ml_dtypes
    bf = ml_dtypes.bfloat16
    x = np.asarray(inputs["hidden_states"], np.float32).reshape(BS, D)
    xt = np.ascontiguousarray(x.T.astype(bf))  # [D, BS] bf16
    Wq = np.asarray(inputs["Wq"], np.float32)
    Wk = np.asarray(inputs["Wk"], np.float32)
    Wv = np.asarray(inputs["Wv"], np.float32)
    Wo = np.asarray(inputs["Wo"], np.float32)
    bq = np.asarray(inputs["bq"], np.float32)
    bk = np.asarray(inputs["bk"], np.float32)
    bv = np.asarray(inputs["bv"], np.float32)

    def wtile(W, js):
        # [D, 128] column slice -> lhsT tile layout [128, 1024]:
        # tile[p, ib*128 + j] = W[ib*128 + p, js.start + j]
        return np.ascontiguousarray(
            W[:, js].reshape(8, 128, 128).transpose(1, 0, 2).reshape(128, D)
            .astype(bf))

    cident = np.eye(128, dtype=np.float32).astype(bf)
    cmask = np.tril(np.ones((128, 128), np.float32)).T.astype(bf)
    cones = np.ones((128, 32), np.float32).astype(bf)
    csel = np.zeros((8, 512), np.float32)
    for r in range(8):
        csel[r, r * 64:(r + 1) * 64] = 1.0
    csel = csel.astype(bf)

    in_maps = []
    for c in range(N_CORES):
        js = slice(c * J, (c + 1) * J)
        in_maps.append({
            "xt": xt,
            "wq": wtile(Wq, js),
            "wk": wtile(Wk, js),
            "wv": wtile(Wv, js),
            "wo": np.ascontiguousarray(Wo[js, :].astype(bf)),
            "bq": np.ascontiguousarray(bq[js]),
            "bk": np.ascontiguousarray(bk[js]),
            "bv": np.ascontiguousarray(bv[js]),
            "cident": cident,
            "cmask": cmask,
            "cones": cones,
            "csel": csel,
        })
    return in_maps


def needs_bias(inputs):
    return any(
        np.any(np.asarray(inputs[k])) for k in ("bq", "bk", "bv"))


def gather_output(results, bo):
    out_t = np.zeros((D, BS), np.float32)
    for c in range(N_CORES):
        out_t += results[c]["out_t"].astype(np.float32)
    out = out_t.T + np.asarray(bo, np.float32)[None, :]
    return out.reshape(B, S, D)


def kernel(**inputs) -> np.ndarray:
    nc = _get_nc(needs_bias(inputs))
    in_maps = make_in_maps(inputs)
    res = run_bass_kernel_spmd(nc, in_maps, core_ids=list(range(N_CORES)))
    return gather_output(res.results, inputs["bo"])


if __name__ == "__main__":
    rng = np.random.default_rng(0)
    ins = {
        "hidden_states": rng.standard_normal((B, S, D), np.float32),
        "Wq": rng.standard_normal((D, D), np.float32) * 0.02,
        "bq": np.zeros(D, np.float32),
        "Wk": rng.standard_normal((D, D), np.float32) * 0.02,
        "bk": np.zeros(D, np.float32),
        "Wv": rng.standard_normal((D, D), np.float32) * 0.02,
        "bv": np.zeros(D, np.float32),
        "Wo": rng.standard_normal((D, D), np.float32) * 0.02,
        "bo": np.zeros(D, np.float32),
    }
    out = kernel(**ins)
    print("out", out.shape, out.dtype, float(np.abs(out).mean()))
